# revision 7
# baseline (speedup 1.0000x reference)
"""ConvCaps (routing-by-agreement) Trainium2 kernel, v2.

Problem: pose (4, 512, 32, 32) f32, W (288, 512, 16) f32 ->
         out (4, 512, 15, 15) f32.

Math (per spatial position l of 15x15=225, per batch b; 900 positions
padded to 1024 = 8 cores x 128 partitions):
  p[l]      : (288, 16)   unfolded poses (k = kk*32 + a)
  votes     : V[k, bd] = sum_c W[k, bd, c] * p[l, k, c]      (288, 512)
  routing (3 iters, softmax over B=32 output caps; D=16):
    u[k,B] = <V[k,B,:], w[B,:]>,  w_i = sum_{j<i} v_j   (w_1 = 0)
    c = softmax_B(u); s[B,:] = sum_k c[k,B] V[k,B,:]; v = squash(s)
  output = v_3  (l, 512)

v2 design notes (vs v1):
  * Votes stored d-major (columns ordered d*32+B, via host-side W column
    permute). All big DVE ops then keep a contiguous innermost B axis,
    so per-(k)/per-(k,B) broadcasts hit middle dims and every large
    tensor_tensor runs in the 2x bf16 perf mode.
  * No TensorReduce on large tensors (1 elem/cycle, no fast modes);
    grouped reductions are pairwise tree adds at 2x, with tail levels
    and the softmax normalize on gpsimd.
  * Pass 1 (s1 = mean_k V) uses a packed 128-deep contraction: 36
    matmuls instead of 288 (PE array fully utilized).
  * Scalar engine (Act) does the PSUM->SBUF bf16 vote copies and exp;
    gpsimd takes small tail ops only (its mult efficiency is poor).
"""

import ml_dtypes
import numpy as np

import concourse.bass as bass
import concourse.tile as tile
from concourse import mybir
from concourse.bass_utils import run_bass_kernel_spmd
from concourse.vector_clock import ScopedClock

# ---- problem constants ----
A, B, K, P, STRIDE, ITERS = 32, 32, 3, 4, 2, 3
C = P * P            # 16
D = P * P            # 16
KK = K * K           # 9
KKA = KK * A         # 288
BD = B * D           # 512
EPS = 1e-8
H = W_IN = 32
OH = (H - K) // STRIDE + 1   # 15
OW = OH                      # 15
L = OH * OW                  # 225
NB = 4                       # batch
NPOS = NB * L                # 900
NCORES = 8
LP = 128                     # positions per core (padded)
NPOS_PAD = NCORES * LP       # 1024

G = 16                       # k-group (unit) size for routing iters
NG = KKA // G                # 18 units
GH = 4                       # PSUM half-group (bank granularity)
KC = 8                       # k's packed per 128-contraction chunk (pass 1)
NCH = KKA // KC              # 36 chunks

F32 = mybir.dt.float32
F32R = mybir.dt.float32r
BF16 = mybir.dt.bfloat16

AX = mybir.AxisListType
OP = mybir.AluOpType
ACT = mybir.ActivationFunctionType


class _ChunkedDrainTileContext(tile.TileContext):
    """Work around a walrus limit of 2 sem-waits per CTRL instruction:
    split the kernel-tail drain's waits across per-processor drains."""

    def _drain_and_barrier(self, tick_clock, wait_clock):
        vclock = tick_clock.global_clock
        observed = ScopedClock()
        for i in range(len(vclock)):
            if vclock[i] > 0:
                partial = ScopedClock()
                partial.require_at_least(None, i, vclock[i])
                d = self.nc.sync.drain()
                wait_clock.add_sem_waits(d.ins, partial, observed)
                observed.update_past(partial)
        drain_inst = self.nc.sync.drain()
        wait_clock.add_sem_waits(
            drain_inst.ins, ScopedClock({None: tick_clock.global_clock}), observed
        )
        self.nc.all_engine_barrier()
        assert self.sems is not None
        popped = self.nc._tile_sem_poison_stack.pop()
        assert popped is self._sem_poison
        self.nc.clear_and_free_semaphores(list(self.sems.allocated().values()))
        self.nc.all_engine_barrier()


def _dB(t):
    """view a [LP, 512] AP as [LP, D, B] (d-major columns)"""
    return t.rearrange("p (d b) -> p d b", b=B)


def _squash_dm(nc, pool, ss, eps_t, out_dtype, tag):
    """ss: [LP, 512] f32 SBUF, d-major. returns v [LP, 512] out_dtype."""
    sq = pool.tile([LP, BD], F32, tag="sq")
    nc.vector.tensor_mul(out=sq, in0=ss, in1=ss)
    # in-place tree-reduce over d (middle dim), f32, 4 levels
    sqv = _dB(sq)
    for h in (8, 4, 2, 1):
        nc.vector.tensor_add(
            out=sqv[:, 0:h, :], in0=sqv[:, 0:h, :], in1=sqv[:, h:2 * h, :]
        )
    n2 = sqv[:, 0, :]                               # [LP, B] strided
    # (n2+eps)^-1/2 = exp(-0.5*ln(n2+eps))
    lg = pool.tile([LP, B], F32, tag="lg")
    nc.scalar.activation(out=lg, in_=n2, func=ACT.Ln, bias=eps_t, scale=1.0)
    np1 = pool.tile([LP, B], F32, tag="np1")
    nc.vector.tensor_scalar_add(out=np1, in0=n2, scalar1=1.0)
    rnp1 = pool.tile([LP, B], F32, tag="rnp1")
    nc.vector.reciprocal(out=rnp1, in_=np1)
    rs = pool.tile([LP, B], F32, tag="rs")
    nc.scalar.activation(out=rs, in_=lg, func=ACT.Exp, bias=0.0, scale=-0.5)
    f1 = pool.tile([LP, B], F32, tag="f1")
    nc.vector.tensor_mul(out=f1, in0=n2, in1=rs)
    fac = pool.tile([LP, B], F32, tag="fac")
    nc.vector.tensor_mul(out=fac, in0=f1, in1=rnp1)
    v = pool.tile([LP, BD], F32 if out_dtype == F32 else BF16, tag=tag)
    nc.vector.tensor_mul(
        out=_dB(v), in0=_dB(ss), in1=fac.unsqueeze(1).to_broadcast([LP, D, B])
    )
    return v


def _build_nc(mm_dtype=F32R):
    nc = bass.Bass("TRN2", target_bir_lowering=False, debug=False)
    # per-k stream for routing iters: cols = [Wdm 512 | poseT 128]
    PW_d = nc.dram_tensor("PW", [C, KKA, BD + LP], BF16, kind="ExternalInput")
    # packed pass-1 stream: 128-contraction chunks, cols = [W2 512 | p2 128]
    PW2_d = nc.dram_tensor("PW2", [128, NCH, BD + LP], BF16, kind="ExternalInput")
    out_d = nc.dram_tensor("vout", [LP, BD], F32, kind="ExternalOutput")
    PW = PW_d.ap()
    PW2 = PW2_d.ap()
    vout = out_d.ap()

    with _ChunkedDrainTileContext(nc) as tc:
        import contextlib

        with contextlib.ExitStack() as ctx:
            wring = ctx.enter_context(tc.tile_pool(name="wring", bufs=4))
            vt_pool = ctx.enter_context(tc.tile_pool(name="vtp", bufs=3))
            um_pool = ctx.enter_context(tc.tile_pool(name="ump", bufs=2))
            sm_pool = ctx.enter_context(tc.tile_pool(name="smp", bufs=2))
            small = ctx.enter_context(tc.tile_pool(name="small", bufs=3))
            keep = ctx.enter_context(tc.tile_pool(name="keep", bufs=1))

            eps_t = keep.tile([LP, 1], F32, tag="eps")
            nc.vector.memset(eps_t, EPS)

            vp_pool = ctx.enter_context(
                tc.tile_pool(name="vp", bufs=2, space="PSUM")
            )

            # ---------- pass 1: s1 = (1/B) sum_k V, packed contraction.
            # Accumulates in one bank of the shared vp ring so iter-2 vote
            # matmuls can start while pass 1 is still streaming. ----
            with tc.tile_pool(name="chring", bufs=4) as chring:
                psum_t = vp_pool.tile([LP, GH, BD], F32, tag="vp")
                psum_s = psum_t[:, 0, :]
                for m in range(NCH):
                    ch = chring.tile([128, BD + LP], BF16, tag="ch")
                    nc.sync.dma_start(out=ch, in_=PW2[:, m, :])
                    nc.tensor.matmul(
                        psum_s,
                        lhsT=ch[:, BD:BD + LP],
                        rhs=ch[:, 0:BD],
                        start=(m == 0),
                        stop=(m == NCH - 1),
                    )
                ss1 = keep.tile([LP, BD], F32, tag="ss1")
                nc.scalar.mul(out=ss1, in_=psum_s, mul=1.0 / B)
            w = _squash_dm(nc, small, ss1, eps_t, BF16, tag="w0")  # v1 == w for iter 2

            # ---------- iters 2..3 ----------
            for it in range(1, ITERS):
                sgk = keep.tile([LP, NG, BD], BF16, tag="sgk")
                vt4s, ums, es, rZs, cs = {}, {}, {}, {}, {}

                def emit_votes_um(g):
                    # --- votes: G k's via four PSUM half-groups ---
                    vt = vt_pool.tile([LP, G, BD], BF16, tag="vt")
                    for half in range(G // GH):
                        k0 = g * G + half * GH
                        wt = wring.tile([C, GH, BD + LP], BF16, tag="wt")
                        nc.sync.dma_start(out=wt, in_=PW[:, k0:k0 + GH, :])
                        vp = vp_pool.tile([LP, GH, BD], F32, tag="vp")
                        for j in range(GH):
                            nc.tensor.matmul(
                                vp[:, j, :],
                                lhsT=wt[:, j, BD:BD + LP],
                                rhs=wt[:, j, 0:BD],
                                start=True,
                                stop=True,
                            )
                        nc.scalar.copy(
                            out=vt[:, half * GH:(half + 1) * GH, :], in_=vp
                        )
                    vt4 = vt.rearrange("p g (d b) -> p g d b", b=B)
                    vt4s[g] = vt4
                    # --- u[k,B] = sum_d V*w : mul + in-place tree over d ---
                    um = um_pool.tile([LP, G, D, B], BF16, tag="um")
                    nc.vector.tensor_mul(
                        out=um,
                        in0=vt4,
                        in1=_dB(w).unsqueeze(1).to_broadcast([LP, G, D, B]),
                    )
                    for h in (8, 4, 2):
                        nc.vector.tensor_add(
                            out=um[:, :, 0:h, :],
                            in0=um[:, :, 0:h, :],
                            in1=um[:, :, h:2 * h, :],
                        )
                    nc.gpsimd.tensor_add(
                        out=um[:, :, 0, :], in0=um[:, :, 0, :], in1=um[:, :, 1, :]
                    )
                    ums[g] = um
                    e = small.tile([LP, G, B], F32, tag="e")
                    nc.scalar.activation(out=e, in_=um[:, :, 0, :], func=ACT.Exp)
                    es[g] = e

                def emit_softmax(g):
                    e = es.pop(g)
                    Z = small.tile([LP, G], F32, tag="Z")
                    nc.vector.tensor_reduce(out=Z, in_=e, axis=AX.X, op=OP.add)
                    rZ = small.tile([LP, G], F32, tag="rZ")
                    nc.vector.reciprocal(out=rZ, in_=Z)
                    c = small.tile([LP, G, B], BF16, tag="c")
                    nc.gpsimd.tensor_mul(
                        out=c, in0=e, in1=rZ.unsqueeze(2).to_broadcast([LP, G, B])
                    )
                    cs[g] = c

                def emit_sm(g):
                    vt4 = vt4s.pop(g)
                    c = cs.pop(g)
                    ums.pop(g, None)
                    # s partial: sm = V*c in [p, d, g, B] order (c bcast on
                    # the outer dim keeps the 2x DVE mode), tree over g
                    sm = sm_pool.tile([LP, D, G, B], BF16, tag="sm")
                    nc.vector.tensor_mul(
                        out=sm,
                        in0=vt4.rearrange("p g d b -> p d g b"),
                        in1=c.unsqueeze(1).to_broadcast([LP, D, G, B]),
                    )
                    for h in (8, 4, 2):
                        nc.vector.tensor_add(
                            out=sm[:, :, 0:h, :],
                            in0=sm[:, :, 0:h, :],
                            in1=sm[:, :, h:2 * h, :],
                        )
                    eng_sg = nc.vector if g >= NG - 2 else nc.gpsimd
                    eng_sg.tensor_add(
                        out=_dB(sgk[:, g, :]), in0=sm[:, :, 0, :], in1=sm[:, :, 1, :]
                    )

                for g in range(NG + 2):
                    if g < NG:
                        emit_votes_um(g)
                    if 0 <= g - 1 < NG:
                        emit_softmax(g - 1)
                    if 0 <= g - 2 < NG:
                        emit_sm(g - 2)

                # end-of-iter pairwise tree over the NG=18 unit partials
                nc.vector.tensor_add(
                    out=sgk[:, 0:9, :], in0=sgk[:, 0:9, :], in1=sgk[:, 9:18, :]
                )
                nc.vector.tensor_add(
                    out=sgk[:, 0:4, :], in0=sgk[:, 0:4, :], in1=sgk[:, 4:8, :]
                )
                nc.vector.tensor_add(
                    out=sgk[:, 0:2, :], in0=sgk[:, 0:2, :], in1=sgk[:, 2:4, :]
                )
                nc.vector.tensor_add(
                    out=sgk[:, 0, :], in0=sgk[:, 0, :], in1=sgk[:, 1, :]
                )
                s_acc = keep.tile([LP, BD], F32, tag=f"sacc{it}")
                nc.vector.tensor_add(
                    out=s_acc, in0=sgk[:, 0, :], in1=sgk[:, 8, :]
                )

                if it < ITERS - 1:
                    v_it = _squash_dm(nc, small, s_acc, eps_t, BF16, tag=f"v{it}")
                    w_new = keep.tile([LP, BD], BF16, tag=f"w{it}")
                    nc.vector.tensor_add(out=w_new, in0=w, in1=v_it)
                    w = w_new
                else:
                    v_it = _squash_dm(nc, small, s_acc, eps_t, F32, tag=f"v{it}")
                    nc.sync.dma_start(out=vout[:, :], in_=v_it)
    _split_excess_waits(nc)
    return nc


def _host_prep(pose, W):
    """unfold + shard + build the two packed DMA streams per core."""
    pose = np.asarray(pose, dtype=np.float32)
    W = np.asarray(W, dtype=np.float32)
    b = pose.shape[0]
    cols = np.empty((b, A * C, KK, OH, OW), dtype=np.float32)
    for ki in range(K):
        for kj in range(K):
            cols[:, :, ki * K + kj] = pose[
                :, :, ki:ki + STRIDE * (OH - 1) + 1:STRIDE,
                kj:kj + STRIDE * (OW - 1) + 1:STRIDE,
            ]
    # (b, A, C, KK, l) -> (b, l, KK, A, C) -> (npos, KKA, C)
    p = cols.reshape(b, A, C, KK, L).transpose(0, 4, 3, 1, 2).reshape(
        NPOS, KKA, C
    )
    p_pad = np.zeros((NPOS_PAD, KKA, C), dtype=np.float32)
    p_pad[:NPOS] = p
    # W d-major: Wdm[k, d*32+B, c] = W[k, B*16+d, c]
    Wdm = W.reshape(KKA, B, D, C).transpose(0, 2, 1, 3).reshape(KKA, BD, C)
    Wt = Wdm.transpose(2, 0, 1)                      # [C, KKA, BD]
    # packed pass-1 W chunks: W2[(j*16+c), m, dB] = Wdm[m*KC+j, dB, c]
    W2 = Wdm.reshape(NCH, KC, BD, C).transpose(0, 1, 3, 2).reshape(
        NCH, KC * C, BD
    ).transpose(1, 0, 2)                             # [128, NCH, BD]
    in_maps = []
    for i in range(NCORES):
        sh = p_pad[i * LP:(i + 1) * LP]              # [LP, KKA, C]
        pT = sh.transpose(2, 1, 0)                   # [C, KKA, LP]
        PW = np.empty((C, KKA, BD + LP), dtype=ml_dtypes.bfloat16)
        PW[:, :, :BD] = Wt
        PW[:, :, BD:] = pT
        # p2[(j*16+c), m, l] = p[l, m*KC+j, c]
        p2 = sh.reshape(LP, NCH, KC, C).transpose(1, 2, 3, 0).reshape(
            NCH, KC * C, LP
        ).transpose(1, 0, 2)                         # [128, NCH, LP]
        PW2 = np.empty((128, NCH, BD + LP), dtype=ml_dtypes.bfloat16)
        PW2[:, :, :BD] = W2
        PW2[:, :, BD:] = p2
        in_maps.append({"PW": PW, "PW2": PW2})
    return in_maps


def _gather(results):
    v = np.concatenate([r["vout"] for r in results], axis=0)  # [1024, 512]
    # d-major columns (d*32+B) -> reference layout (B*16+d)
    v = v[:NPOS].reshape(NPOS, D, B).transpose(0, 2, 1).reshape(NB, L, BD)
    v = v.transpose(0, 2, 1)
    return np.ascontiguousarray(v.reshape(NB, BD, OH, OW), dtype=np.float32)


def _split_excess_waits(nc, max_waits=1):
    """walrus (CoreV2/V3) accepts at most 2 sync-wait commands per
    compute instruction and 1 per DMA; hoist excess waits onto NOPs
    just before, same engine."""
    n_split = 0
    for f in nc.m.functions:
        for bb in f.blocks:
            il = bb.instructions
            out = []
            changed = False
            for inst in il:
                lim = max_waits
                si = inst.sync_info
                if si is not None and si.on_wait and len(si.on_wait) > lim:
                    waits = list(si.on_wait)
                    excess, kept = waits[:-lim], waits[-lim:]
                    for i in range(0, len(excess), max_waits):
                        nop = mybir.InstNoOp(
                            name=f"{inst.name}-w{i}",
                            sync_info=mybir.SyncInfo(
                                on_wait=excess[i:i + max_waits], on_update=[]
                            ),
                            bass_nofuse=True,
                            engine=inst.engine,
                        )
                        out.append(nop)
                        n_split += 1
                    inst.sync_info = mybir.SyncInfo(
                        on_wait=kept, on_update=list(si.on_update or [])
                    )
                    changed = True
                out.append(inst)
            if changed:
                bb.instructions = out
    return n_split


_NC_CACHE = {}


def _get_nc(mm_dtype=F32R):
    key = str(mm_dtype)
    if key not in _NC_CACHE:
        _NC_CACHE[key] = _build_nc(mm_dtype)
    return _NC_CACHE[key]


def _run(pose, W, trace=False, mm_dtype=F32R):
    nc = _get_nc(mm_dtype)
    in_maps = _host_prep(pose, W)
    res = run_bass_kernel_spmd(
        nc, in_maps, core_ids=list(range(NCORES)), trace=trace
    )
    return _gather(res.results), res


def kernel(pose, W):
    out, _ = _run(pose, W)
    return out


# revision 8
# speedup vs baseline: 1.0344x; 1.0344x over previous
"""ConvCaps (routing-by-agreement) Trainium2 kernel, v2.

Problem: pose (4, 512, 32, 32) f32, W (288, 512, 16) f32 ->
         out (4, 512, 15, 15) f32.

Math (per spatial position l of 15x15=225, per batch b; 900 positions
padded to 1024 = 8 cores x 128 partitions):
  p[l]      : (288, 16)   unfolded poses (k = kk*32 + a)
  votes     : V[k, bd] = sum_c W[k, bd, c] * p[l, k, c]      (288, 512)
  routing (3 iters, softmax over B=32 output caps; D=16):
    u[k,B] = <V[k,B,:], w[B,:]>,  w_i = sum_{j<i} v_j   (w_1 = 0)
    c = softmax_B(u); s[B,:] = sum_k c[k,B] V[k,B,:]; v = squash(s)
  output = v_3  (l, 512)

v2 design notes (vs v1):
  * Votes stored d-major (columns ordered d*32+B, via host-side W column
    permute). All big DVE ops then keep a contiguous innermost B axis,
    so per-(k)/per-(k,B) broadcasts hit middle dims and every large
    tensor_tensor runs in the 2x bf16 perf mode.
  * No TensorReduce on large tensors (1 elem/cycle, no fast modes);
    grouped reductions are pairwise tree adds at 2x, with tail levels
    and the softmax normalize on gpsimd.
  * Pass 1 (s1 = mean_k V) uses a packed 128-deep contraction: 36
    matmuls instead of 288 (PE array fully utilized).
  * Scalar engine (Act) does the PSUM->SBUF bf16 vote copies and exp;
    gpsimd takes small tail ops only (its mult efficiency is poor).
"""

import ml_dtypes
import numpy as np

import concourse.bass as bass
import concourse.tile as tile
from concourse import mybir
from concourse.bass_utils import run_bass_kernel_spmd
from concourse.vector_clock import ScopedClock

# ---- problem constants ----
A, B, K, P, STRIDE, ITERS = 32, 32, 3, 4, 2, 3
C = P * P            # 16
D = P * P            # 16
KK = K * K           # 9
KKA = KK * A         # 288
BD = B * D           # 512
EPS = 1e-8
H = W_IN = 32
OH = (H - K) // STRIDE + 1   # 15
OW = OH                      # 15
L = OH * OW                  # 225
NB = 4                       # batch
NPOS = NB * L                # 900
NCORES = 8
LP = 128                     # positions per core (padded)
NPOS_PAD = NCORES * LP       # 1024

G = 16                       # k-group (unit) size for routing iters
NG = KKA // G                # 18 units
GH = 4                       # PSUM half-group (bank granularity)
KC = 8                       # k's packed per 128-contraction chunk (pass 1)
NCH = KKA // KC              # 36 chunks

F32 = mybir.dt.float32
F32R = mybir.dt.float32r
BF16 = mybir.dt.bfloat16

AX = mybir.AxisListType
OP = mybir.AluOpType
ACT = mybir.ActivationFunctionType


class _ChunkedDrainTileContext(tile.TileContext):
    """Work around a walrus limit of 2 sem-waits per CTRL instruction:
    split the kernel-tail drain's waits across per-processor drains."""

    def _drain_and_barrier(self, tick_clock, wait_clock):
        vclock = tick_clock.global_clock
        observed = ScopedClock()
        for i in range(len(vclock)):
            if vclock[i] > 0:
                partial = ScopedClock()
                partial.require_at_least(None, i, vclock[i])
                d = self.nc.sync.drain()
                wait_clock.add_sem_waits(d.ins, partial, observed)
                observed.update_past(partial)
        drain_inst = self.nc.sync.drain()
        wait_clock.add_sem_waits(
            drain_inst.ins, ScopedClock({None: tick_clock.global_clock}), observed
        )
        self.nc.all_engine_barrier()
        assert self.sems is not None
        popped = self.nc._tile_sem_poison_stack.pop()
        assert popped is self._sem_poison
        self.nc.clear_and_free_semaphores(list(self.sems.allocated().values()))
        self.nc.all_engine_barrier()


def _dB(t):
    """view a [LP, 512] AP as [LP, D, B] (d-major columns)"""
    return t.rearrange("p (d b) -> p d b", b=B)


def _squash_dm(nc, pool, ss, eps_t, out_dtype, tag):
    """ss: [LP, 512] f32 SBUF, d-major. returns v [LP, 512] out_dtype."""
    sq = pool.tile([LP, BD], F32, tag="sq")
    nc.vector.tensor_mul(out=sq, in0=ss, in1=ss)
    # in-place tree-reduce over d (middle dim), f32, 4 levels
    sqv = _dB(sq)
    for h in (8, 4, 2, 1):
        nc.vector.tensor_add(
            out=sqv[:, 0:h, :], in0=sqv[:, 0:h, :], in1=sqv[:, h:2 * h, :]
        )
    n2 = sqv[:, 0, :]                               # [LP, B] strided
    # (n2+eps)^-1/2 = exp(-0.5*ln(n2+eps))
    lg = pool.tile([LP, B], F32, tag="lg")
    nc.scalar.activation(out=lg, in_=n2, func=ACT.Ln, bias=eps_t, scale=1.0)
    np1 = pool.tile([LP, B], F32, tag="np1")
    nc.vector.tensor_scalar_add(out=np1, in0=n2, scalar1=1.0)
    rnp1 = pool.tile([LP, B], F32, tag="rnp1")
    nc.vector.reciprocal(out=rnp1, in_=np1)
    rs = pool.tile([LP, B], F32, tag="rs")
    nc.scalar.activation(out=rs, in_=lg, func=ACT.Exp, bias=0.0, scale=-0.5)
    f1 = pool.tile([LP, B], F32, tag="f1")
    nc.vector.tensor_mul(out=f1, in0=n2, in1=rs)
    fac = pool.tile([LP, B], F32, tag="fac")
    nc.vector.tensor_mul(out=fac, in0=f1, in1=rnp1)
    v = pool.tile([LP, BD], F32 if out_dtype == F32 else BF16, tag=tag)
    nc.vector.tensor_mul(
        out=_dB(v), in0=_dB(ss), in1=fac.unsqueeze(1).to_broadcast([LP, D, B])
    )
    return v


def _build_nc(mm_dtype=F32R):
    nc = bass.Bass("TRN2", target_bir_lowering=False, debug=False)
    # per-k stream for routing iters: cols = [Wdm 512 | poseT 128]
    PW_d = nc.dram_tensor("PW", [C, KKA, BD + LP], BF16, kind="ExternalInput")
    # packed pass-1 stream: 128-contraction chunks, cols = [W2 512 | p2 128]
    PW2_d = nc.dram_tensor("PW2", [128, NCH, BD + LP], BF16, kind="ExternalInput")
    out_d = nc.dram_tensor("vout", [LP, BD], F32, kind="ExternalOutput")
    PW = PW_d.ap()
    PW2 = PW2_d.ap()
    vout = out_d.ap()

    with _ChunkedDrainTileContext(nc) as tc:
        import contextlib

        with contextlib.ExitStack() as ctx:
            wring = ctx.enter_context(tc.tile_pool(name="wring", bufs=4))
            vt_pool = ctx.enter_context(tc.tile_pool(name="vtp", bufs=3))
            um_pool = ctx.enter_context(tc.tile_pool(name="ump", bufs=2))
            sm_pool = ctx.enter_context(tc.tile_pool(name="smp", bufs=2))
            small = ctx.enter_context(tc.tile_pool(name="small", bufs=3))
            keep = ctx.enter_context(tc.tile_pool(name="keep", bufs=1))

            eps_t = keep.tile([LP, 1], F32, tag="eps")
            nc.vector.memset(eps_t, EPS)

            # ---------- pass 1: s1 = (1/B) sum_k V, packed contraction ----
            with tc.tile_pool(name="s1", bufs=1, space="PSUM") as s1_pool, \
                    tc.tile_pool(name="chring", bufs=4) as chring:
                psum_s = s1_pool.tile([LP, BD], F32)
                for m in range(NCH):
                    ch = chring.tile([128, BD + LP], BF16, tag="ch")
                    nc.sync.dma_start(out=ch, in_=PW2[:, m, :])
                    nc.tensor.matmul(
                        psum_s,
                        lhsT=ch[:, BD:BD + LP],
                        rhs=ch[:, 0:BD],
                        start=(m == 0),
                        stop=(m == NCH - 1),
                    )
                ss1 = keep.tile([LP, BD], F32, tag="ss1")
                nc.scalar.mul(out=ss1, in_=psum_s, mul=1.0 / B)
            w = _squash_dm(nc, small, ss1, eps_t, BF16, tag="w0")  # v1 == w for iter 2

            vp_pool = ctx.enter_context(
                tc.tile_pool(name="vp", bufs=2, space="PSUM")
            )

            # ---------- iters 2..3 ----------
            for it in range(1, ITERS):
                sgk = keep.tile([LP, NG, BD], BF16, tag="sgk")
                vt4s, ums, es, rZs, cs = {}, {}, {}, {}, {}

                def emit_votes_um(g):
                    # --- votes: G k's via four PSUM half-groups ---
                    vt = vt_pool.tile([LP, G, BD], BF16, tag="vt")
                    for half in range(G // GH):
                        k0 = g * G + half * GH
                        wt = wring.tile([C, GH, BD + LP], BF16, tag="wt")
                        nc.sync.dma_start(out=wt, in_=PW[:, k0:k0 + GH, :])
                        vp = vp_pool.tile([LP, GH, BD], F32, tag="vp")
                        for j in range(GH):
                            nc.tensor.matmul(
                                vp[:, j, :],
                                lhsT=wt[:, j, BD:BD + LP],
                                rhs=wt[:, j, 0:BD],
                                start=True,
                                stop=True,
                            )
                        nc.scalar.copy(
                            out=vt[:, half * GH:(half + 1) * GH, :], in_=vp
                        )
                    vt4 = vt.rearrange("p g (d b) -> p g d b", b=B)
                    vt4s[g] = vt4
                    # --- u[k,B] = sum_d V*w : mul + in-place tree over d ---
                    um = um_pool.tile([LP, G, D, B], BF16, tag="um")
                    nc.vector.tensor_mul(
                        out=um,
                        in0=vt4,
                        in1=_dB(w).unsqueeze(1).to_broadcast([LP, G, D, B]),
                    )
                    for h in (8, 4, 2):
                        nc.vector.tensor_add(
                            out=um[:, :, 0:h, :],
                            in0=um[:, :, 0:h, :],
                            in1=um[:, :, h:2 * h, :],
                        )
                    nc.vector.tensor_add(
                        out=um[:, :, 0, :], in0=um[:, :, 0, :], in1=um[:, :, 1, :]
                    )
                    ums[g] = um
                    e = small.tile([LP, G, B], F32, tag="e")
                    nc.scalar.activation(out=e, in_=um[:, :, 0, :], func=ACT.Exp)
                    es[g] = e

                def emit_softmax(g):
                    e = es.pop(g)
                    Z = small.tile([LP, G], F32, tag="Z")
                    nc.vector.tensor_reduce(out=Z, in_=e, axis=AX.X, op=OP.add)
                    rZ = small.tile([LP, G], F32, tag="rZ")
                    nc.vector.reciprocal(out=rZ, in_=Z)
                    c = small.tile([LP, G, B], BF16, tag="c")
                    nc.gpsimd.tensor_mul(
                        out=c, in0=e, in1=rZ.unsqueeze(2).to_broadcast([LP, G, B])
                    )
                    cs[g] = c

                def emit_sm(g):
                    vt4 = vt4s.pop(g)
                    c = cs.pop(g)
                    ums.pop(g, None)
                    # s partial: sm = V*c in [p, d, g, B] order (c bcast on
                    # the outer dim keeps the 2x DVE mode), tree over g
                    sm = sm_pool.tile([LP, D, G, B], BF16, tag="sm")
                    nc.vector.tensor_mul(
                        out=sm,
                        in0=vt4.rearrange("p g d b -> p d g b"),
                        in1=c.unsqueeze(1).to_broadcast([LP, D, G, B]),
                    )
                    for h in (8, 4, 2):
                        nc.vector.tensor_add(
                            out=sm[:, :, 0:h, :],
                            in0=sm[:, :, 0:h, :],
                            in1=sm[:, :, h:2 * h, :],
                        )
                    nc.vector.tensor_add(
                        out=_dB(sgk[:, g, :]), in0=sm[:, :, 0, :], in1=sm[:, :, 1, :]
                    )

                for g in range(NG + 2):
                    if g < NG:
                        emit_votes_um(g)
                    if 0 <= g - 1 < NG:
                        emit_softmax(g - 1)
                    if 0 <= g - 2 < NG:
                        emit_sm(g - 2)

                # end-of-iter pairwise tree over the NG=18 unit partials
                nc.vector.tensor_add(
                    out=sgk[:, 0:9, :], in0=sgk[:, 0:9, :], in1=sgk[:, 9:18, :]
                )
                nc.vector.tensor_add(
                    out=sgk[:, 0:4, :], in0=sgk[:, 0:4, :], in1=sgk[:, 4:8, :]
                )
                nc.vector.tensor_add(
                    out=sgk[:, 0:2, :], in0=sgk[:, 0:2, :], in1=sgk[:, 2:4, :]
                )
                nc.vector.tensor_add(
                    out=sgk[:, 0, :], in0=sgk[:, 0, :], in1=sgk[:, 1, :]
                )
                s_acc = keep.tile([LP, BD], F32, tag=f"sacc{it}")
                nc.vector.tensor_add(
                    out=s_acc, in0=sgk[:, 0, :], in1=sgk[:, 8, :]
                )

                if it < ITERS - 1:
                    v_it = _squash_dm(nc, small, s_acc, eps_t, BF16, tag=f"v{it}")
                    w_new = keep.tile([LP, BD], BF16, tag=f"w{it}")
                    nc.vector.tensor_add(out=w_new, in0=w, in1=v_it)
                    w = w_new
                else:
                    v_it = _squash_dm(nc, small, s_acc, eps_t, F32, tag=f"v{it}")
                    nc.sync.dma_start(out=vout[:, :], in_=v_it)
    _split_excess_waits(nc)
    return nc


def _host_prep(pose, W):
    """unfold + shard + build the two packed DMA streams per core."""
    pose = np.asarray(pose, dtype=np.float32)
    W = np.asarray(W, dtype=np.float32)
    b = pose.shape[0]
    cols = np.empty((b, A * C, KK, OH, OW), dtype=np.float32)
    for ki in range(K):
        for kj in range(K):
            cols[:, :, ki * K + kj] = pose[
                :, :, ki:ki + STRIDE * (OH - 1) + 1:STRIDE,
                kj:kj + STRIDE * (OW - 1) + 1:STRIDE,
            ]
    # (b, A, C, KK, l) -> (b, l, KK, A, C) -> (npos, KKA, C)
    p = cols.reshape(b, A, C, KK, L).transpose(0, 4, 3, 1, 2).reshape(
        NPOS, KKA, C
    )
    p_pad = np.zeros((NPOS_PAD, KKA, C), dtype=np.float32)
    p_pad[:NPOS] = p
    # W d-major: Wdm[k, d*32+B, c] = W[k, B*16+d, c]
    Wdm = W.reshape(KKA, B, D, C).transpose(0, 2, 1, 3).reshape(KKA, BD, C)
    Wt = Wdm.transpose(2, 0, 1)                      # [C, KKA, BD]
    # packed pass-1 W chunks: W2[(j*16+c), m, dB] = Wdm[m*KC+j, dB, c]
    W2 = Wdm.reshape(NCH, KC, BD, C).transpose(0, 1, 3, 2).reshape(
        NCH, KC * C, BD
    ).transpose(1, 0, 2)                             # [128, NCH, BD]
    in_maps = []
    for i in range(NCORES):
        sh = p_pad[i * LP:(i + 1) * LP]              # [LP, KKA, C]
        pT = sh.transpose(2, 1, 0)                   # [C, KKA, LP]
        PW = np.empty((C, KKA, BD + LP), dtype=ml_dtypes.bfloat16)
        PW[:, :, :BD] = Wt
        PW[:, :, BD:] = pT
        # p2[(j*16+c), m, l] = p[l, m*KC+j, c]
        p2 = sh.reshape(LP, NCH, KC, C).transpose(1, 2, 3, 0).reshape(
            NCH, KC * C, LP
        ).transpose(1, 0, 2)                         # [128, NCH, LP]
        PW2 = np.empty((128, NCH, BD + LP), dtype=ml_dtypes.bfloat16)
        PW2[:, :, :BD] = W2
        PW2[:, :, BD:] = p2
        in_maps.append({"PW": PW, "PW2": PW2})
    return in_maps


def _gather(results):
    v = np.concatenate([r["vout"] for r in results], axis=0)  # [1024, 512]
    # d-major columns (d*32+B) -> reference layout (B*16+d)
    v = v[:NPOS].reshape(NPOS, D, B).transpose(0, 2, 1).reshape(NB, L, BD)
    v = v.transpose(0, 2, 1)
    return np.ascontiguousarray(v.reshape(NB, BD, OH, OW), dtype=np.float32)


def _split_excess_waits(nc, max_waits=1):
    """walrus (CoreV2/V3) accepts at most 2 sync-wait commands per
    compute instruction and 1 per DMA; hoist excess waits onto NOPs
    just before, same engine."""
    n_split = 0
    for f in nc.m.functions:
        for bb in f.blocks:
            il = bb.instructions
            out = []
            changed = False
            for inst in il:
                lim = max_waits
                si = inst.sync_info
                if si is not None and si.on_wait and len(si.on_wait) > lim:
                    waits = list(si.on_wait)
                    excess, kept = waits[:-lim], waits[-lim:]
                    for i in range(0, len(excess), max_waits):
                        nop = mybir.InstNoOp(
                            name=f"{inst.name}-w{i}",
                            sync_info=mybir.SyncInfo(
                                on_wait=excess[i:i + max_waits], on_update=[]
                            ),
                            bass_nofuse=True,
                            engine=inst.engine,
                        )
                        out.append(nop)
                        n_split += 1
                    inst.sync_info = mybir.SyncInfo(
                        on_wait=kept, on_update=list(si.on_update or [])
                    )
                    changed = True
                out.append(inst)
            if changed:
                bb.instructions = out
    return n_split


_NC_CACHE = {}


def _get_nc(mm_dtype=F32R):
    key = str(mm_dtype)
    if key not in _NC_CACHE:
        _NC_CACHE[key] = _build_nc(mm_dtype)
    return _NC_CACHE[key]


def _run(pose, W, trace=False, mm_dtype=F32R):
    nc = _get_nc(mm_dtype)
    in_maps = _host_prep(pose, W)
    res = run_bass_kernel_spmd(
        nc, in_maps, core_ids=list(range(NCORES)), trace=trace
    )
    return _gather(res.results), res


def kernel(pose, W):
    out, _ = _run(pose, W)
    return out


# revision 9
# speedup vs baseline: 1.0483x; 1.0135x over previous
"""ConvCaps (routing-by-agreement) Trainium2 kernel, v2.

Problem: pose (4, 512, 32, 32) f32, W (288, 512, 16) f32 ->
         out (4, 512, 15, 15) f32.

Math (per spatial position l of 15x15=225, per batch b; 900 positions
padded to 1024 = 8 cores x 128 partitions):
  p[l]      : (288, 16)   unfolded poses (k = kk*32 + a)
  votes     : V[k, bd] = sum_c W[k, bd, c] * p[l, k, c]      (288, 512)
  routing (3 iters, softmax over B=32 output caps; D=16):
    u[k,B] = <V[k,B,:], w[B,:]>,  w_i = sum_{j<i} v_j   (w_1 = 0)
    c = softmax_B(u); s[B,:] = sum_k c[k,B] V[k,B,:]; v = squash(s)
  output = v_3  (l, 512)

v2 design notes (vs v1):
  * Votes stored d-major (columns ordered d*32+B, via host-side W column
    permute). All big DVE ops then keep a contiguous innermost B axis,
    so per-(k)/per-(k,B) broadcasts hit middle dims and every large
    tensor_tensor runs in the 2x bf16 perf mode.
  * No TensorReduce on large tensors (1 elem/cycle, no fast modes);
    grouped reductions are pairwise tree adds at 2x, with tail levels
    and the softmax normalize on gpsimd.
  * Pass 1 (s1 = mean_k V) uses a packed 128-deep contraction: 36
    matmuls instead of 288 (PE array fully utilized).
  * Scalar engine (Act) does the PSUM->SBUF bf16 vote copies and exp;
    gpsimd takes small tail ops only (its mult efficiency is poor).
"""

import ml_dtypes
import numpy as np

import concourse.bass as bass
import concourse.tile as tile
from concourse import mybir
from concourse.bass_utils import run_bass_kernel_spmd
from concourse.vector_clock import ScopedClock

# ---- problem constants ----
A, B, K, P, STRIDE, ITERS = 32, 32, 3, 4, 2, 3
C = P * P            # 16
D = P * P            # 16
KK = K * K           # 9
KKA = KK * A         # 288
BD = B * D           # 512
EPS = 1e-8
H = W_IN = 32
OH = (H - K) // STRIDE + 1   # 15
OW = OH                      # 15
L = OH * OW                  # 225
NB = 4                       # batch
NPOS = NB * L                # 900
NCORES = 8
LP = 128                     # positions per core (padded)
NPOS_PAD = NCORES * LP       # 1024

G = 16                       # k-group (unit) size for routing iters
NG = KKA // G                # 18 units
GH = 4                       # PSUM half-group (bank granularity)
KC = 8                       # k's packed per 128-contraction chunk (pass 1)
NCH = KKA // KC              # 36 chunks

F32 = mybir.dt.float32
F32R = mybir.dt.float32r
BF16 = mybir.dt.bfloat16

AX = mybir.AxisListType
OP = mybir.AluOpType
ACT = mybir.ActivationFunctionType


class _ChunkedDrainTileContext(tile.TileContext):
    """Work around a walrus limit of 2 sem-waits per CTRL instruction:
    split the kernel-tail drain's waits across per-processor drains."""

    def _drain_and_barrier(self, tick_clock, wait_clock):
        vclock = tick_clock.global_clock
        observed = ScopedClock()
        for i in range(len(vclock)):
            if vclock[i] > 0:
                partial = ScopedClock()
                partial.require_at_least(None, i, vclock[i])
                d = self.nc.sync.drain()
                wait_clock.add_sem_waits(d.ins, partial, observed)
                observed.update_past(partial)
        drain_inst = self.nc.sync.drain()
        wait_clock.add_sem_waits(
            drain_inst.ins, ScopedClock({None: tick_clock.global_clock}), observed
        )
        self.nc.all_engine_barrier()
        assert self.sems is not None
        popped = self.nc._tile_sem_poison_stack.pop()
        assert popped is self._sem_poison
        self.nc.clear_and_free_semaphores(list(self.sems.allocated().values()))
        self.nc.all_engine_barrier()


def _dB(t):
    """view a [LP, 512] AP as [LP, D, B] (d-major columns)"""
    return t.rearrange("p (d b) -> p d b", b=B)


def _squash_dm(nc, pool, ss, eps_t, out_dtype, tag):
    """ss: [LP, 512] f32 SBUF, d-major. returns v [LP, 512] out_dtype."""
    sq = pool.tile([LP, BD], F32, tag="sq")
    nc.vector.tensor_mul(out=sq, in0=ss, in1=ss)
    # in-place tree-reduce over d (middle dim), f32, 4 levels
    sqv = _dB(sq)
    for h in (8, 4, 2, 1):
        nc.vector.tensor_add(
            out=sqv[:, 0:h, :], in0=sqv[:, 0:h, :], in1=sqv[:, h:2 * h, :]
        )
    n2 = sqv[:, 0, :]                               # [LP, B] strided
    # (n2+eps)^-1/2 = exp(-0.5*ln(n2+eps))
    lg = pool.tile([LP, B], F32, tag="lg")
    nc.scalar.activation(out=lg, in_=n2, func=ACT.Ln, bias=eps_t, scale=1.0)
    np1 = pool.tile([LP, B], F32, tag="np1")
    nc.vector.tensor_scalar_add(out=np1, in0=n2, scalar1=1.0)
    rnp1 = pool.tile([LP, B], F32, tag="rnp1")
    nc.vector.reciprocal(out=rnp1, in_=np1)
    rs = pool.tile([LP, B], F32, tag="rs")
    nc.scalar.activation(out=rs, in_=lg, func=ACT.Exp, bias=0.0, scale=-0.5)
    f1 = pool.tile([LP, B], F32, tag="f1")
    nc.vector.tensor_mul(out=f1, in0=n2, in1=rs)
    fac = pool.tile([LP, B], F32, tag="fac")
    nc.vector.tensor_mul(out=fac, in0=f1, in1=rnp1)
    v = pool.tile([LP, BD], F32 if out_dtype == F32 else BF16, tag=tag)
    nc.vector.tensor_mul(
        out=_dB(v), in0=_dB(ss), in1=fac.unsqueeze(1).to_broadcast([LP, D, B])
    )
    return v


def _build_nc(mm_dtype=F32R):
    nc = bass.Bass("TRN2", target_bir_lowering=False, debug=False)
    # per-k stream for routing iters: cols = [Wdm 512 | poseT 128]
    PW_d = nc.dram_tensor("PW", [C, KKA, BD + LP], BF16, kind="ExternalInput")
    # packed pass-1 stream: 128-contraction chunks, cols = [W2 512 | p2 128]
    PW2_d = nc.dram_tensor("PW2", [128, NCH, BD + LP], BF16, kind="ExternalInput")
    out_d = nc.dram_tensor("vout", [LP, BD], F32, kind="ExternalOutput")
    PW = PW_d.ap()
    PW2 = PW2_d.ap()
    vout = out_d.ap()

    with _ChunkedDrainTileContext(nc) as tc:
        import contextlib

        with contextlib.ExitStack() as ctx:
            wring = ctx.enter_context(tc.tile_pool(name="wring", bufs=4))
            vt_pool = ctx.enter_context(tc.tile_pool(name="vtp", bufs=3))
            um_pool = ctx.enter_context(tc.tile_pool(name="ump", bufs=2))
            sm_pool = ctx.enter_context(tc.tile_pool(name="smp", bufs=2))
            small = ctx.enter_context(tc.tile_pool(name="small", bufs=3))
            keep = ctx.enter_context(tc.tile_pool(name="keep", bufs=1))

            eps_t = keep.tile([LP, 1], F32, tag="eps")
            nc.vector.memset(eps_t, EPS)

            # ---------- pass 1: s1 = (1/B) sum_k V, packed contraction ----
            with tc.tile_pool(name="s1", bufs=1, space="PSUM") as s1_pool, \
                    tc.tile_pool(name="chring", bufs=4) as chring:
                psum_s = s1_pool.tile([LP, BD], F32)
                for m in range(0, NCH, 2):
                    ch = chring.tile([128, 2, BD + LP], BF16, tag="ch")
                    nc.sync.dma_start(out=ch, in_=PW2[:, m:m + 2, :])
                    for j in range(2):
                        nc.tensor.matmul(
                            psum_s,
                            lhsT=ch[:, j, BD:BD + LP],
                            rhs=ch[:, j, 0:BD],
                            start=(m + j == 0),
                            stop=(m + j == NCH - 1),
                        )
                ss1 = keep.tile([LP, BD], F32, tag="ss1")
                nc.scalar.mul(out=ss1, in_=psum_s, mul=1.0 / B)
            w = _squash_dm(nc, small, ss1, eps_t, BF16, tag="w0")  # v1 == w for iter 2

            vp_pool = ctx.enter_context(
                tc.tile_pool(name="vp", bufs=2, space="PSUM")
            )

            # ---------- iters 2..3 ----------
            for it in range(1, ITERS):
                sgk = keep.tile([LP, NG, BD], BF16, tag="sgk")
                vt4s, ums, es, rZs, cs = {}, {}, {}, {}, {}

                def emit_votes_um(g):
                    # --- votes: G k's via four PSUM half-groups ---
                    vt = vt_pool.tile([LP, G, BD], BF16, tag="vt")
                    for half in range(G // GH):
                        k0 = g * G + half * GH
                        wt = wring.tile([C, GH, BD + LP], BF16, tag="wt")
                        nc.sync.dma_start(out=wt, in_=PW[:, k0:k0 + GH, :])
                        vp = vp_pool.tile([LP, GH, BD], F32, tag="vp")
                        for j in range(GH):
                            nc.tensor.matmul(
                                vp[:, j, :],
                                lhsT=wt[:, j, BD:BD + LP],
                                rhs=wt[:, j, 0:BD],
                                start=True,
                                stop=True,
                            )
                        nc.scalar.copy(
                            out=vt[:, half * GH:(half + 1) * GH, :], in_=vp
                        )
                    vt4 = vt.rearrange("p g (d b) -> p g d b", b=B)
                    vt4s[g] = vt4
                    # --- u[k,B] = sum_d V*w : mul + in-place tree over d ---
                    um = um_pool.tile([LP, G, D, B], BF16, tag="um")
                    nc.vector.tensor_mul(
                        out=um,
                        in0=vt4,
                        in1=_dB(w).unsqueeze(1).to_broadcast([LP, G, D, B]),
                    )
                    for h in (8, 4, 2):
                        nc.vector.tensor_add(
                            out=um[:, :, 0:h, :],
                            in0=um[:, :, 0:h, :],
                            in1=um[:, :, h:2 * h, :],
                        )
                    nc.vector.tensor_add(
                        out=um[:, :, 0, :], in0=um[:, :, 0, :], in1=um[:, :, 1, :]
                    )
                    ums[g] = um
                    e = small.tile([LP, G, B], F32, tag="e")
                    nc.scalar.activation(out=e, in_=um[:, :, 0, :], func=ACT.Exp)
                    es[g] = e

                def emit_softmax(g):
                    e = es.pop(g)
                    Z = small.tile([LP, G], F32, tag="Z")
                    nc.vector.tensor_reduce(out=Z, in_=e, axis=AX.X, op=OP.add)
                    rZ = small.tile([LP, G], F32, tag="rZ")
                    nc.vector.reciprocal(out=rZ, in_=Z)
                    c = small.tile([LP, G, B], BF16, tag="c")
                    nc.gpsimd.tensor_mul(
                        out=c, in0=e, in1=rZ.unsqueeze(2).to_broadcast([LP, G, B])
                    )
                    cs[g] = c

                def emit_sm(g):
                    vt4 = vt4s.pop(g)
                    c = cs.pop(g)
                    ums.pop(g, None)
                    # s partial: sm = V*c in [p, d, g, B] order (c bcast on
                    # the outer dim keeps the 2x DVE mode), tree over g
                    sm = sm_pool.tile([LP, D, G, B], BF16, tag="sm")
                    nc.vector.tensor_mul(
                        out=sm,
                        in0=vt4.rearrange("p g d b -> p d g b"),
                        in1=c.unsqueeze(1).to_broadcast([LP, D, G, B]),
                    )
                    for h in (8, 4, 2):
                        nc.vector.tensor_add(
                            out=sm[:, :, 0:h, :],
                            in0=sm[:, :, 0:h, :],
                            in1=sm[:, :, h:2 * h, :],
                        )
                    nc.vector.tensor_add(
                        out=_dB(sgk[:, g, :]), in0=sm[:, :, 0, :], in1=sm[:, :, 1, :]
                    )

                for g in range(NG + 2):
                    if g < NG:
                        emit_votes_um(g)
                    if 0 <= g - 1 < NG:
                        emit_softmax(g - 1)
                    if 0 <= g - 2 < NG:
                        emit_sm(g - 2)

                # end-of-iter pairwise tree over the NG=18 unit partials
                nc.vector.tensor_add(
                    out=sgk[:, 0:9, :], in0=sgk[:, 0:9, :], in1=sgk[:, 9:18, :]
                )
                nc.vector.tensor_add(
                    out=sgk[:, 0:4, :], in0=sgk[:, 0:4, :], in1=sgk[:, 4:8, :]
                )
                nc.vector.tensor_add(
                    out=sgk[:, 0:2, :], in0=sgk[:, 0:2, :], in1=sgk[:, 2:4, :]
                )
                nc.vector.tensor_add(
                    out=sgk[:, 0, :], in0=sgk[:, 0, :], in1=sgk[:, 1, :]
                )
                s_acc = keep.tile([LP, BD], F32, tag=f"sacc{it}")
                nc.vector.tensor_add(
                    out=s_acc, in0=sgk[:, 0, :], in1=sgk[:, 8, :]
                )

                if it < ITERS - 1:
                    v_it = _squash_dm(nc, small, s_acc, eps_t, BF16, tag=f"v{it}")
                    w_new = keep.tile([LP, BD], BF16, tag=f"w{it}")
                    nc.vector.tensor_add(out=w_new, in0=w, in1=v_it)
                    w = w_new
                else:
                    v_it = _squash_dm(nc, small, s_acc, eps_t, F32, tag=f"v{it}")
                    nc.sync.dma_start(out=vout[:, :], in_=v_it)
    _split_excess_waits(nc)
    return nc


def _host_prep(pose, W):
    """unfold + shard + build the two packed DMA streams per core."""
    pose = np.asarray(pose, dtype=np.float32)
    W = np.asarray(W, dtype=np.float32)
    b = pose.shape[0]
    cols = np.empty((b, A * C, KK, OH, OW), dtype=np.float32)
    for ki in range(K):
        for kj in range(K):
            cols[:, :, ki * K + kj] = pose[
                :, :, ki:ki + STRIDE * (OH - 1) + 1:STRIDE,
                kj:kj + STRIDE * (OW - 1) + 1:STRIDE,
            ]
    # (b, A, C, KK, l) -> (b, l, KK, A, C) -> (npos, KKA, C)
    p = cols.reshape(b, A, C, KK, L).transpose(0, 4, 3, 1, 2).reshape(
        NPOS, KKA, C
    )
    p_pad = np.zeros((NPOS_PAD, KKA, C), dtype=np.float32)
    p_pad[:NPOS] = p
    # W d-major: Wdm[k, d*32+B, c] = W[k, B*16+d, c]
    Wdm = W.reshape(KKA, B, D, C).transpose(0, 2, 1, 3).reshape(KKA, BD, C)
    Wt = Wdm.transpose(2, 0, 1)                      # [C, KKA, BD]
    # packed pass-1 W chunks: W2[(j*16+c), m, dB] = Wdm[m*KC+j, dB, c]
    W2 = Wdm.reshape(NCH, KC, BD, C).transpose(0, 1, 3, 2).reshape(
        NCH, KC * C, BD
    ).transpose(1, 0, 2)                             # [128, NCH, BD]
    in_maps = []
    for i in range(NCORES):
        sh = p_pad[i * LP:(i + 1) * LP]              # [LP, KKA, C]
        pT = sh.transpose(2, 1, 0)                   # [C, KKA, LP]
        PW = np.empty((C, KKA, BD + LP), dtype=ml_dtypes.bfloat16)
        PW[:, :, :BD] = Wt
        PW[:, :, BD:] = pT
        # p2[(j*16+c), m, l] = p[l, m*KC+j, c]
        p2 = sh.reshape(LP, NCH, KC, C).transpose(1, 2, 3, 0).reshape(
            NCH, KC * C, LP
        ).transpose(1, 0, 2)                         # [128, NCH, LP]
        PW2 = np.empty((128, NCH, BD + LP), dtype=ml_dtypes.bfloat16)
        PW2[:, :, :BD] = W2
        PW2[:, :, BD:] = p2
        in_maps.append({"PW": PW, "PW2": PW2})
    return in_maps


def _gather(results):
    v = np.concatenate([r["vout"] for r in results], axis=0)  # [1024, 512]
    # d-major columns (d*32+B) -> reference layout (B*16+d)
    v = v[:NPOS].reshape(NPOS, D, B).transpose(0, 2, 1).reshape(NB, L, BD)
    v = v.transpose(0, 2, 1)
    return np.ascontiguousarray(v.reshape(NB, BD, OH, OW), dtype=np.float32)


def _split_excess_waits(nc, max_waits=1):
    """walrus (CoreV2/V3) accepts at most 2 sync-wait commands per
    compute instruction and 1 per DMA; hoist excess waits onto NOPs
    just before, same engine."""
    n_split = 0
    for f in nc.m.functions:
        for bb in f.blocks:
            il = bb.instructions
            out = []
            changed = False
            for inst in il:
                lim = max_waits
                si = inst.sync_info
                if si is not None and si.on_wait and len(si.on_wait) > lim:
                    waits = list(si.on_wait)
                    excess, kept = waits[:-lim], waits[-lim:]
                    for i in range(0, len(excess), max_waits):
                        nop = mybir.InstNoOp(
                            name=f"{inst.name}-w{i}",
                            sync_info=mybir.SyncInfo(
                                on_wait=excess[i:i + max_waits], on_update=[]
                            ),
                            bass_nofuse=True,
                            engine=inst.engine,
                        )
                        out.append(nop)
                        n_split += 1
                    inst.sync_info = mybir.SyncInfo(
                        on_wait=kept, on_update=list(si.on_update or [])
                    )
                    changed = True
                out.append(inst)
            if changed:
                bb.instructions = out
    return n_split


_NC_CACHE = {}


def _get_nc(mm_dtype=F32R):
    key = str(mm_dtype)
    if key not in _NC_CACHE:
        _NC_CACHE[key] = _build_nc(mm_dtype)
    return _NC_CACHE[key]


def _run(pose, W, trace=False, mm_dtype=F32R):
    nc = _get_nc(mm_dtype)
    in_maps = _host_prep(pose, W)
    res = run_bass_kernel_spmd(
        nc, in_maps, core_ids=list(range(NCORES)), trace=trace
    )
    return _gather(res.results), res


def kernel(pose, W):
    out, _ = _run(pose, W)
    return out


# revision 10
# speedup vs baseline: 1.0595x; 1.0106x over previous
"""ConvCaps (routing-by-agreement) Trainium2 kernel, v2.

Problem: pose (4, 512, 32, 32) f32, W (288, 512, 16) f32 ->
         out (4, 512, 15, 15) f32.

Math (per spatial position l of 15x15=225, per batch b; 900 positions
padded to 1024 = 8 cores x 128 partitions):
  p[l]      : (288, 16)   unfolded poses (k = kk*32 + a)
  votes     : V[k, bd] = sum_c W[k, bd, c] * p[l, k, c]      (288, 512)
  routing (3 iters, softmax over B=32 output caps; D=16):
    u[k,B] = <V[k,B,:], w[B,:]>,  w_i = sum_{j<i} v_j   (w_1 = 0)
    c = softmax_B(u); s[B,:] = sum_k c[k,B] V[k,B,:]; v = squash(s)
  output = v_3  (l, 512)

v2 design notes (vs v1):
  * Votes stored d-major (columns ordered d*32+B, via host-side W column
    permute). All big DVE ops then keep a contiguous innermost B axis,
    so per-(k)/per-(k,B) broadcasts hit middle dims and every large
    tensor_tensor runs in the 2x bf16 perf mode.
  * No TensorReduce on large tensors (1 elem/cycle, no fast modes);
    grouped reductions are pairwise tree adds at 2x, with tail levels
    and the softmax normalize on gpsimd.
  * Pass 1 (s1 = mean_k V) uses a packed 128-deep contraction: 36
    matmuls instead of 288 (PE array fully utilized).
  * Scalar engine (Act) does the PSUM->SBUF bf16 vote copies and exp;
    gpsimd takes small tail ops only (its mult efficiency is poor).
"""

import ml_dtypes
import numpy as np

import concourse.bass as bass
import concourse.tile as tile
from concourse import mybir
from concourse.bass_utils import run_bass_kernel_spmd
from concourse.vector_clock import ScopedClock

# ---- problem constants ----
A, B, K, P, STRIDE, ITERS = 32, 32, 3, 4, 2, 3
C = P * P            # 16
D = P * P            # 16
KK = K * K           # 9
KKA = KK * A         # 288
BD = B * D           # 512
EPS = 1e-8
H = W_IN = 32
OH = (H - K) // STRIDE + 1   # 15
OW = OH                      # 15
L = OH * OW                  # 225
NB = 4                       # batch
NPOS = NB * L                # 900
NCORES = 8
LP = 128                     # positions per core (padded)
NPOS_PAD = NCORES * LP       # 1024

G = 16                       # k-group (unit) size for routing iters
NG = KKA // G                # 18 units
GH = 4                       # PSUM half-group (bank granularity)
KC = 8                       # k's packed per 128-contraction chunk (pass 1)
NCH = KKA // KC              # 36 chunks

F32 = mybir.dt.float32
F32R = mybir.dt.float32r
BF16 = mybir.dt.bfloat16

AX = mybir.AxisListType
OP = mybir.AluOpType
ACT = mybir.ActivationFunctionType


class _ChunkedDrainTileContext(tile.TileContext):
    """Work around a walrus limit of 2 sem-waits per CTRL instruction:
    split the kernel-tail drain's waits across per-processor drains."""

    def _drain_and_barrier(self, tick_clock, wait_clock):
        vclock = tick_clock.global_clock
        observed = ScopedClock()
        for i in range(len(vclock)):
            if vclock[i] > 0:
                partial = ScopedClock()
                partial.require_at_least(None, i, vclock[i])
                d = self.nc.sync.drain()
                wait_clock.add_sem_waits(d.ins, partial, observed)
                observed.update_past(partial)
        drain_inst = self.nc.sync.drain()
        wait_clock.add_sem_waits(
            drain_inst.ins, ScopedClock({None: tick_clock.global_clock}), observed
        )
        self.nc.all_engine_barrier()
        assert self.sems is not None
        popped = self.nc._tile_sem_poison_stack.pop()
        assert popped is self._sem_poison
        self.nc.clear_and_free_semaphores(list(self.sems.allocated().values()))
        self.nc.all_engine_barrier()


def _dB(t):
    """view a [LP, 512] AP as [LP, D, B] (d-major columns)"""
    return t.rearrange("p (d b) -> p d b", b=B)


def _squash_dm(nc, pool, ss, eps_t, out_dtype, tag):
    """ss: [LP, 512] f32 SBUF, d-major. returns v [LP, 512] out_dtype."""
    sq = pool.tile([LP, BD], F32, tag="sq")
    nc.vector.tensor_mul(out=sq, in0=ss, in1=ss)
    # in-place tree-reduce over d (middle dim), f32, 4 levels
    sqv = _dB(sq)
    for h in (8, 4, 2, 1):
        nc.vector.tensor_add(
            out=sqv[:, 0:h, :], in0=sqv[:, 0:h, :], in1=sqv[:, h:2 * h, :]
        )
    n2 = sqv[:, 0, :]                               # [LP, B] strided
    sq2 = pool.tile([LP, B], F32, tag="lg")
    nc.scalar.activation(out=sq2, in_=n2, func=ACT.Sqrt, bias=eps_t, scale=1.0)
    np1 = pool.tile([LP, B], F32, tag="np1")
    nc.vector.tensor_scalar_add(out=np1, in0=n2, scalar1=1.0)
    rnp1 = pool.tile([LP, B], F32, tag="rnp1")
    nc.vector.reciprocal(out=rnp1, in_=np1)
    f1 = pool.tile([LP, B], F32, tag="f1")
    nc.vector.tensor_mul(out=f1, in0=n2, in1=rnp1)
    rs = pool.tile([LP, B], F32, tag="rs")
    nc.vector.reciprocal(out=rs, in_=sq2)
    fac = pool.tile([LP, B], F32, tag="fac")
    nc.vector.tensor_mul(out=fac, in0=f1, in1=rs)
    v = pool.tile([LP, BD], F32 if out_dtype == F32 else BF16, tag=tag)
    nc.vector.tensor_mul(
        out=_dB(v), in0=_dB(ss), in1=fac.unsqueeze(1).to_broadcast([LP, D, B])
    )
    return v


def _build_nc(mm_dtype=F32R):
    nc = bass.Bass("TRN2", target_bir_lowering=False, debug=False)
    # per-k stream for routing iters: cols = [Wdm 512 | poseT 128]
    PW_d = nc.dram_tensor("PW", [C, KKA, BD + LP], BF16, kind="ExternalInput")
    # packed pass-1 stream: 128-contraction chunks, cols = [W2 512 | p2 128]
    PW2_d = nc.dram_tensor("PW2", [128, NCH, BD + LP], BF16, kind="ExternalInput")
    out_d = nc.dram_tensor("vout", [LP, BD], F32, kind="ExternalOutput")
    PW = PW_d.ap()
    PW2 = PW2_d.ap()
    vout = out_d.ap()

    with _ChunkedDrainTileContext(nc) as tc:
        import contextlib

        with contextlib.ExitStack() as ctx:
            wring = ctx.enter_context(tc.tile_pool(name="wring", bufs=4))
            vt_pool = ctx.enter_context(tc.tile_pool(name="vtp", bufs=3))
            um_pool = ctx.enter_context(tc.tile_pool(name="ump", bufs=2))
            sm_pool = ctx.enter_context(tc.tile_pool(name="smp", bufs=2))
            small = ctx.enter_context(tc.tile_pool(name="small", bufs=3))
            keep = ctx.enter_context(tc.tile_pool(name="keep", bufs=1))

            eps_t = keep.tile([LP, 1], F32, tag="eps")
            nc.vector.memset(eps_t, EPS)

            # ---------- pass 1: s1 = (1/B) sum_k V, packed contraction ----
            with tc.tile_pool(name="s1", bufs=1, space="PSUM") as s1_pool, \
                    tc.tile_pool(name="chring", bufs=4) as chring:
                psum_s = s1_pool.tile([LP, BD], F32)
                for m in range(0, NCH, 4):
                    ch = chring.tile([128, 4, BD + LP], BF16, tag="ch")
                    nc.sync.dma_start(out=ch, in_=PW2[:, m:m + 4, :])
                    for j in range(4):
                        nc.tensor.matmul(
                            psum_s,
                            lhsT=ch[:, j, BD:BD + LP],
                            rhs=ch[:, j, 0:BD],
                            start=(m + j == 0),
                            stop=(m + j == NCH - 1),
                        )
                ss1 = keep.tile([LP, BD], F32, tag="ss1")
                nc.scalar.mul(out=ss1, in_=psum_s, mul=1.0 / B)
            w = _squash_dm(nc, small, ss1, eps_t, BF16, tag="w0")  # v1 == w for iter 2

            vp_pool = ctx.enter_context(
                tc.tile_pool(name="vp", bufs=2, space="PSUM")
            )

            # ---------- iters 2..3 ----------
            for it in range(1, ITERS):
                sgk = keep.tile([LP, NG, BD], BF16, tag="sgk")
                vt4s, ums, es, rZs, cs = {}, {}, {}, {}, {}

                def emit_votes_um(g):
                    # --- votes: G k's via four PSUM half-groups ---
                    vt = vt_pool.tile([LP, G, BD], BF16, tag="vt")
                    for half in range(G // GH):
                        k0 = g * G + half * GH
                        wt = wring.tile([C, GH, BD + LP], BF16, tag="wt")
                        nc.sync.dma_start(out=wt, in_=PW[:, k0:k0 + GH, :])
                        vp = vp_pool.tile([LP, GH, BD], F32, tag="vp")
                        for j in range(GH):
                            nc.tensor.matmul(
                                vp[:, j, :],
                                lhsT=wt[:, j, BD:BD + LP],
                                rhs=wt[:, j, 0:BD],
                                start=True,
                                stop=True,
                            )
                        nc.scalar.copy(
                            out=vt[:, half * GH:(half + 1) * GH, :], in_=vp
                        )
                    vt4 = vt.rearrange("p g (d b) -> p g d b", b=B)
                    vt4s[g] = vt4
                    # --- u[k,B] = sum_d V*w : mul + in-place tree over d ---
                    um = um_pool.tile([LP, G, D, B], BF16, tag="um")
                    nc.vector.tensor_mul(
                        out=um,
                        in0=vt4,
                        in1=_dB(w).unsqueeze(1).to_broadcast([LP, G, D, B]),
                    )
                    for h in (8, 4, 2):
                        nc.vector.tensor_add(
                            out=um[:, :, 0:h, :],
                            in0=um[:, :, 0:h, :],
                            in1=um[:, :, h:2 * h, :],
                        )
                    nc.vector.tensor_add(
                        out=um[:, :, 0, :], in0=um[:, :, 0, :], in1=um[:, :, 1, :]
                    )
                    ums[g] = um
                    e = small.tile([LP, G, B], F32, tag="e")
                    nc.scalar.activation(out=e, in_=um[:, :, 0, :], func=ACT.Exp)
                    es[g] = e

                def emit_softmax(g):
                    e = es.pop(g)
                    Z = small.tile([LP, G], F32, tag="Z")
                    nc.vector.tensor_reduce(out=Z, in_=e, axis=AX.X, op=OP.add)
                    rZ = small.tile([LP, G], F32, tag="rZ")
                    nc.vector.reciprocal(out=rZ, in_=Z)
                    c = small.tile([LP, G, B], BF16, tag="c")
                    nc.gpsimd.tensor_mul(
                        out=c, in0=e, in1=rZ.unsqueeze(2).to_broadcast([LP, G, B])
                    )
                    cs[g] = c

                def emit_sm(g):
                    vt4 = vt4s.pop(g)
                    c = cs.pop(g)
                    ums.pop(g, None)
                    # s partial: sm = V*c in [p, d, g, B] order (c bcast on
                    # the outer dim keeps the 2x DVE mode), tree over g
                    sm = sm_pool.tile([LP, D, G, B], BF16, tag="sm")
                    nc.vector.tensor_mul(
                        out=sm,
                        in0=vt4.rearrange("p g d b -> p d g b"),
                        in1=c.unsqueeze(1).to_broadcast([LP, D, G, B]),
                    )
                    for h in (8, 4, 2):
                        nc.vector.tensor_add(
                            out=sm[:, :, 0:h, :],
                            in0=sm[:, :, 0:h, :],
                            in1=sm[:, :, h:2 * h, :],
                        )
                    nc.vector.tensor_add(
                        out=_dB(sgk[:, g, :]), in0=sm[:, :, 0, :], in1=sm[:, :, 1, :]
                    )

                for g in range(NG + 2):
                    if g < NG:
                        emit_votes_um(g)
                    if 0 <= g - 1 < NG:
                        emit_softmax(g - 1)
                    if 0 <= g - 2 < NG:
                        emit_sm(g - 2)

                # end-of-iter pairwise tree over the NG=18 unit partials
                nc.vector.tensor_add(
                    out=sgk[:, 0:9, :], in0=sgk[:, 0:9, :], in1=sgk[:, 9:18, :]
                )
                nc.vector.tensor_add(
                    out=sgk[:, 0:4, :], in0=sgk[:, 0:4, :], in1=sgk[:, 4:8, :]
                )
                nc.vector.tensor_add(
                    out=sgk[:, 0:2, :], in0=sgk[:, 0:2, :], in1=sgk[:, 2:4, :]
                )
                nc.vector.tensor_add(
                    out=sgk[:, 0, :], in0=sgk[:, 0, :], in1=sgk[:, 1, :]
                )
                s_acc = keep.tile([LP, BD], F32, tag=f"sacc{it}")
                nc.vector.tensor_add(
                    out=s_acc, in0=sgk[:, 0, :], in1=sgk[:, 8, :]
                )

                if it < ITERS - 1:
                    v_it = _squash_dm(nc, small, s_acc, eps_t, BF16, tag=f"v{it}")
                    w_new = keep.tile([LP, BD], BF16, tag=f"w{it}")
                    nc.vector.tensor_add(out=w_new, in0=w, in1=v_it)
                    w = w_new
                else:
                    v_it = _squash_dm(nc, small, s_acc, eps_t, F32, tag=f"v{it}")
                    nc.sync.dma_start(out=vout[:, :], in_=v_it)
    _split_excess_waits(nc)
    return nc


def _host_prep(pose, W):
    """unfold + shard + build the two packed DMA streams per core."""
    pose = np.asarray(pose, dtype=np.float32)
    W = np.asarray(W, dtype=np.float32)
    b = pose.shape[0]
    cols = np.empty((b, A * C, KK, OH, OW), dtype=np.float32)
    for ki in range(K):
        for kj in range(K):
            cols[:, :, ki * K + kj] = pose[
                :, :, ki:ki + STRIDE * (OH - 1) + 1:STRIDE,
                kj:kj + STRIDE * (OW - 1) + 1:STRIDE,
            ]
    # (b, A, C, KK, l) -> (b, l, KK, A, C) -> (npos, KKA, C)
    p = cols.reshape(b, A, C, KK, L).transpose(0, 4, 3, 1, 2).reshape(
        NPOS, KKA, C
    )
    p_pad = np.zeros((NPOS_PAD, KKA, C), dtype=np.float32)
    p_pad[:NPOS] = p
    # W d-major: Wdm[k, d*32+B, c] = W[k, B*16+d, c]
    Wdm = W.reshape(KKA, B, D, C).transpose(0, 2, 1, 3).reshape(KKA, BD, C)
    Wt = Wdm.transpose(2, 0, 1)                      # [C, KKA, BD]
    # packed pass-1 W chunks: W2[(j*16+c), m, dB] = Wdm[m*KC+j, dB, c]
    W2 = Wdm.reshape(NCH, KC, BD, C).transpose(0, 1, 3, 2).reshape(
        NCH, KC * C, BD
    ).transpose(1, 0, 2)                             # [128, NCH, BD]
    in_maps = []
    for i in range(NCORES):
        sh = p_pad[i * LP:(i + 1) * LP]              # [LP, KKA, C]
        pT = sh.transpose(2, 1, 0)                   # [C, KKA, LP]
        PW = np.empty((C, KKA, BD + LP), dtype=ml_dtypes.bfloat16)
        PW[:, :, :BD] = Wt
        PW[:, :, BD:] = pT
        # p2[(j*16+c), m, l] = p[l, m*KC+j, c]
        p2 = sh.reshape(LP, NCH, KC, C).transpose(1, 2, 3, 0).reshape(
            NCH, KC * C, LP
        ).transpose(1, 0, 2)                         # [128, NCH, LP]
        PW2 = np.empty((128, NCH, BD + LP), dtype=ml_dtypes.bfloat16)
        PW2[:, :, :BD] = W2
        PW2[:, :, BD:] = p2
        in_maps.append({"PW": PW, "PW2": PW2})
    return in_maps


def _gather(results):
    v = np.concatenate([r["vout"] for r in results], axis=0)  # [1024, 512]
    # d-major columns (d*32+B) -> reference layout (B*16+d)
    v = v[:NPOS].reshape(NPOS, D, B).transpose(0, 2, 1).reshape(NB, L, BD)
    v = v.transpose(0, 2, 1)
    return np.ascontiguousarray(v.reshape(NB, BD, OH, OW), dtype=np.float32)


def _split_excess_waits(nc, max_waits=1):
    """walrus (CoreV2/V3) accepts at most 2 sync-wait commands per
    compute instruction and 1 per DMA; hoist excess waits onto NOPs
    just before, same engine."""
    n_split = 0
    for f in nc.m.functions:
        for bb in f.blocks:
            il = bb.instructions
            out = []
            changed = False
            for inst in il:
                lim = max_waits
                si = inst.sync_info
                if si is not None and si.on_wait and len(si.on_wait) > lim:
                    waits = list(si.on_wait)
                    excess, kept = waits[:-lim], waits[-lim:]
                    for i in range(0, len(excess), max_waits):
                        nop = mybir.InstNoOp(
                            name=f"{inst.name}-w{i}",
                            sync_info=mybir.SyncInfo(
                                on_wait=excess[i:i + max_waits], on_update=[]
                            ),
                            bass_nofuse=True,
                            engine=inst.engine,
                        )
                        out.append(nop)
                        n_split += 1
                    inst.sync_info = mybir.SyncInfo(
                        on_wait=kept, on_update=list(si.on_update or [])
                    )
                    changed = True
                out.append(inst)
            if changed:
                bb.instructions = out
    return n_split


_NC_CACHE = {}


def _get_nc(mm_dtype=F32R):
    key = str(mm_dtype)
    if key not in _NC_CACHE:
        _NC_CACHE[key] = _build_nc(mm_dtype)
    return _NC_CACHE[key]


def _run(pose, W, trace=False, mm_dtype=F32R):
    nc = _get_nc(mm_dtype)
    in_maps = _host_prep(pose, W)
    res = run_bass_kernel_spmd(
        nc, in_maps, core_ids=list(range(NCORES)), trace=trace
    )
    return _gather(res.results), res


def kernel(pose, W):
    out, _ = _run(pose, W)
    return out
